# revision 1
# baseline (speedup 1.0000x reference)
"""LLaMA-style MLP (gate/up/silu/down) on 8 Trainium2 NeuronCores.

Strategy: data-parallel over tokens (8192 tokens -> 1024/core), bf16
matmuls with fp32 PSUM accumulation, no collectives. Host pre-permutes
all operands into partition-major layouts so the device kernel performs
no transposes:

  x  [B,S,D] -> per core xt  [n_tn, 128, D/128, TB]   xt[tn,p,ds,t] = x[tok, ds*128+p]
  Wg [F,D]   ->          wg  [F/128, 128, D/128, 128] wg[fm,p,ds,f] = Wg[fm*128+f, ds*128+p]
  Wu [F,D]   ->          wu  (same as wg)
  Wd [D,F]   ->          wd  [D/128, 128, F/128, 128] wd[dm,p,fs,d] = Wd[dm*128+d, fs*128+p]
  out        <-          y   [D/128, 128, T]          y[dm,p,t] = out[tok, dm*128+p]

Per token block TB=512: gate/up projections accumulate over D in PSUM
(matmul lhsT=W tile [d128,f128], rhs=x tile [d128,t512]), SiLU on the
scalar engine, gate*up on the vector engine into an SBUF-resident
h [128, F/128, TB] (bf16), then the down projection accumulates over F
(lhsT=Wd tile [f128,d128], rhs=h tile [f128,t512]) and streams y out.
"""

import os
import sys

sys.path.insert(0, "/opt/trn_rl_repo")

from contextlib import ExitStack

import numpy as np
import ml_dtypes

import concourse.bass as bass  # noqa: F401
import concourse.tile as tile
import concourse.mybir as mybir
from concourse import bacc
from concourse.bass_utils import run_bass_kernel_spmd

BF16 = mybir.dt.bfloat16
F32 = mybir.dt.float32

# Problem shape (hardcoded per the task contract).
B, S, D, F = 4, 2048, 4096, 11008
N_CORES = 8
T_CORE = (B * S) // N_CORES  # tokens per core
TB = 512                     # token block (one PSUM bank of fp32)

LAST_RUN = {}


def build_module(T=T_CORE, tb=TB, d=D, f=F):
    """Build the single-core Bass module (same program on all 8 cores)."""
    n_tn = T // tb
    n_ds = d // 128
    n_fm = f // 128
    n_dm = d // 128

    nc = bacc.Bacc("TRN2", target_bir_lowering=False, debug=False)
    xt = nc.dram_tensor("xt", [n_tn, 128, n_ds, tb], BF16, kind="ExternalInput").ap()
    wg = nc.dram_tensor("wg", [n_fm, 128, n_ds, 128], BF16, kind="ExternalInput").ap()
    wu = nc.dram_tensor("wu", [n_fm, 128, n_ds, 128], BF16, kind="ExternalInput").ap()
    wd = nc.dram_tensor("wd", [n_dm, 128, n_fm, 128], BF16, kind="ExternalInput").ap()
    y = nc.dram_tensor("y", [n_dm, 128, T], F32, kind="ExternalOutput").ap()

    with tile.TileContext(nc) as tc, ExitStack() as ctx:
        xpool = ctx.enter_context(tc.tile_pool(name="x", bufs=1))
        wpool = ctx.enter_context(tc.tile_pool(name="w", bufs=2))
        wdpool = ctx.enter_context(tc.tile_pool(name="wdp", bufs=2))
        hpool = ctx.enter_context(tc.tile_pool(name="h", bufs=1))
        spool = ctx.enter_context(tc.tile_pool(name="s", bufs=2))
        ypool = ctx.enter_context(tc.tile_pool(name="y", bufs=2))
        psum = ctx.enter_context(tc.tile_pool(name="psum", bufs=8, space="PSUM"))

        for tn in range(n_tn):
            x_sb = xpool.tile([128, n_ds, tb], BF16, tag="x")
            nc.sync.dma_start(x_sb[:], xt[tn])
            h_sb = hpool.tile([128, n_fm, tb], BF16, tag="h")

            # Stage A: gate/up projection + silu + mul, one 128-row slab of F
            # at a time.
            for fm in range(n_fm):
                wg_sb = wpool.tile([128, n_ds, 128], BF16, tag="w")
                nc.sync.dma_start(wg_sb[:], wg[fm])
                wu_sb = wpool.tile([128, n_ds, 128], BF16, tag="w")
                nc.sync.dma_start(wu_sb[:], wu[fm])

                psg = psum.tile([128, tb], F32, tag="ps")
                for ds in range(n_ds):
                    nc.tensor.matmul(
                        psg[:], wg_sb[:, ds], x_sb[:, ds],
                        start=(ds == 0), stop=(ds == n_ds - 1),
                    )
                psu = psum.tile([128, tb], F32, tag="ps")
                for ds in range(n_ds):
                    nc.tensor.matmul(
                        psu[:], wu_sb[:, ds], x_sb[:, ds],
                        start=(ds == 0), stop=(ds == n_ds - 1),
                    )

                sg = spool.tile([128, tb], BF16, tag="sg")
                nc.scalar.activation(sg[:], psg[:], mybir.ActivationFunctionType.Silu)
                nc.vector.tensor_mul(h_sb[:, fm], sg[:], psu[:])

            # Stage B: down projection, contracting over all of F.
            for dm in range(n_dm):
                wd_sb = wdpool.tile([128, n_fm, 128], BF16, tag="wd")
                nc.sync.dma_start(wd_sb[:], wd[dm])
                psy = psum.tile([128, tb], F32, tag="ps")
                for fm in range(n_fm):
                    nc.tensor.matmul(
                        psy[:], wd_sb[:, fm], h_sb[:, fm],
                        start=(fm == 0), stop=(fm == n_fm - 1),
                    )
                y_sb = ypool.tile([128, tb], F32, tag="y")
                nc.vector.tensor_copy(y_sb[:], psy[:])
                nc.sync.dma_start(y[dm, :, tn * tb:(tn + 1) * tb], y_sb[:])

    nc.compile()
    return nc


def _prep_inputs(x, W_gate, W_up, W_down, T=T_CORE, tb=TB, d=D, f=F,
                 n_cores=N_CORES):
    """Host-side shard + permute + bf16 cast. Returns in_maps for spmd run."""
    n_tn = T // tb
    n_ds = d // 128
    n_fm = f // 128
    n_dm = d // 128

    bf = ml_dtypes.bfloat16
    tokens = np.ascontiguousarray(np.asarray(x, dtype=np.float32).reshape(-1, d))

    # wg[fm, p, ds, f] = Wg[fm*128+f, ds*128+p]
    wg_np = np.ascontiguousarray(
        np.asarray(W_gate, dtype=np.float32).astype(bf)
        .reshape(n_fm, 128, n_ds, 128).transpose(0, 3, 2, 1))
    wu_np = np.ascontiguousarray(
        np.asarray(W_up, dtype=np.float32).astype(bf)
        .reshape(n_fm, 128, n_ds, 128).transpose(0, 3, 2, 1))
    # wd[dm, p, fs, dcol] = Wd[dm*128+dcol, fs*128+p]
    wd_np = np.ascontiguousarray(
        np.asarray(W_down, dtype=np.float32).astype(bf)
        .reshape(n_dm, 128, n_fm, 128).transpose(0, 3, 2, 1))

    in_maps = []
    for c in range(n_cores):
        xc = tokens[c * T:(c + 1) * T]  # [T, d]
        # xt[tn, p, ds, t] = xc[tn*tb + t, ds*128 + p]
        xt_np = np.ascontiguousarray(
            xc.astype(bf).reshape(n_tn, tb, n_ds, 128).transpose(0, 3, 2, 1))
        in_maps.append({"xt": xt_np, "wg": wg_np, "wu": wu_np, "wd": wd_np})
    return in_maps


def _postprocess(results, T=T_CORE, d=D, n_cores=N_CORES):
    """y[dm, p, t] per core -> full [B, S, D] float32."""
    outs = []
    for c in range(n_cores):
        yc = results[c]["y"]  # [n_dm, 128, T]
        outs.append(yc.transpose(2, 0, 1).reshape(T, d))
    return np.concatenate(outs, axis=0)


def kernel(x, W_gate, W_up, W_down):
    import time

    if "nc" not in LAST_RUN:
        t0 = time.perf_counter()
        LAST_RUN["nc"] = build_module()
        LAST_RUN["build_s"] = time.perf_counter() - t0
    nc = LAST_RUN["nc"]

    t0 = time.perf_counter()
    in_maps = _prep_inputs(x, W_gate, W_up, W_down)
    LAST_RUN["prep_s"] = time.perf_counter() - t0

    t0 = time.perf_counter()
    res = run_bass_kernel_spmd(nc, in_maps, core_ids=list(range(N_CORES)))
    LAST_RUN["run_s"] = time.perf_counter() - t0
    LAST_RUN["results"] = res

    out = _postprocess(res.results)
    return out.reshape(B, S, D)

